# revision 45
# baseline (speedup 1.0000x reference)
"""Trainium2 Bass kernel for nn_BatchRNN — v2e: v2c with fp8 eye (cheaper LDWEIGHTS).

Each core runs ONE direction x 16 sequences, split into 2 free-running
chains of 8 sequences offset by half a step, so chain B's matmul round
executes while chain A's activations/vector ops run (and vice versa),
hiding the inter-engine dependency latency of the serial LSTM recurrence.

Per chain-step: 16 stationary-Wh matmuls (N=8) accumulate Wh*h into PSUM;
a DVE add folds the precomputed input projection xg in; one sigmoid over
all 4 gates; 3 DVE ops for the c update; tanh; 2 DVE ops write h (bf16)
split low/high so the next step's k=0 matmuls start early.

BatchNorm + padding mask are folded into host preprocessing; projection
matmul groups for chunk ch+1 are interleaved into the scan of chunk ch.
"""

import sys

sys.path.insert(0, "/opt/trn_rl_repo")

import numpy as np

B, T, D, H = 64, 1024, 512, 256
H4 = 4 * H
EPS = 1e-3
P = 128
S = 16                 # sequences per core
SC = 8                 # sequences per chain
GROUPS = B // S        # 4
KD = D // P            # 4
KH = H // P            # 2
M8 = H4 // P           # 8
TC = 128               # time chunk
NCH = T // TC
SL = 2 * SC            # 16 h cols per chain-step (k-chunk x seq)
GW = M8 * SC           # 64 gate cols per chain-step

_COMPILED = {}
LAST_RESULT = None


def _build_graph(loop_n=None, has_bias=False):
    from concourse import bacc, bass, mybir, tile

    BF = mybir.dt.bfloat16
    F32 = mybir.dt.float32
    AF = mybir.ActivationFunctionType

    nc = bacc.Bacc("TRN2", target_bir_lowering=False, debug=False, num_devices=8)

    F8 = mybir.dt.float8e4
    xT = nc.dram_tensor("xT", [D, 2 * T * SC], BF, kind="ExternalInput").ap()
    wx = nc.dram_tensor("wx", [KD, P, H4], BF, kind="ExternalInput").ap()
    wh = nc.dram_tensor("wh", [KH, P, H4], F8, kind="ExternalInput").ap()
    eye = nc.dram_tensor("eye", [P, P], F8, kind="ExternalInput").ap()
    if has_bias:
        gb = nc.dram_tensor("gb", [P, M8], F32, kind="ExternalInput").ap()
    out = nc.dram_tensor("out", [P, 2 * T * SL], BF, kind="ExternalOutput").ap()

    WCH = TC * SC  # 1024 token-cols per chunk per chain

    with tile.TileContext(nc) as tc:
        with (
            tc.tile_pool(name="const", bufs=1) as const,
            tc.tile_pool(name="state", bufs=1) as state,
            tc.tile_pool(name="xpool", bufs=2) as xpool,
            tc.tile_pool(name="xgpool", bufs=2) as xgpool,
            tc.tile_pool(name="hpool", bufs=2) as hpool,
            tc.tile_pool(name="spool", bufs=3) as spool,
            tc.tile_pool(name="psproj", bufs=2, space="PSUM") as psproj,
            tc.tile_pool(name="psscan", bufs=2, space="PSUM") as psscan,
        ):
            wx_sb = []
            for k in range(KD):
                tw = const.tile([P, H4], BF, tag=f"wx{k}")
                nc.sync.dma_start(tw[:], wx[k])
                wx_sb.append(tw)
            wh_sb = []
            for k in range(KH):
                tw = const.tile([P, H4], F8, tag=f"wh{k}")
                nc.sync.dma_start(tw[:], wh[k])
                wh_sb.append(tw)
            eye_sb = const.tile([P, P], F8, tag="eye")
            nc.sync.dma_start(eye_sb[:], eye[:])
            if has_bias:
                gbt = const.tile([P, M8], F32, tag="gbt")
                nc.sync.dma_start(gbt[:], gb[:])

            # per-chain scan-based cell state (c/2):
            # zt: sigmoid scatter — i at even cols of [0:32], f at evens of
            # [32:64], g at evens of [64:96], o at evens of [96:128]; the
            # scan d0 window zt[31:63] reads [0, f0, 0, f1, ...] (odd cols
            # never written after memset-0)
            zt = [state.tile([P, 8 * SL], F32, tag=f"zt{c}", name=f"zt{c}")
                  for c in range(2)]
            vt = [[state.tile([P, 2 * SL + 2], F32, tag=f"v{c}{i}",
                              name=f"v{c}{i}") for i in range(2)]
                  for c in range(2)]
            tcc = [state.tile([P, SL], F32, tag=f"tc{c}", name=f"tc{c}")
                   for c in range(2)]
            gstep = [0, 0]

            def dma_chunk(ch):
                xin = []
                for c in range(2):
                    row = []
                    for k in range(KD):
                        t = xpool.tile([P, WCH], BF, tag=f"xin{c}_{k}", name=f"xin{c}_{k}")
                        nc.sync.dma_start(
                            t[:],
                            xT[k * P:(k + 1) * P,
                               c * T * SC + ch * WCH: c * T * SC + (ch + 1) * WCH],
                        )
                        row.append(t)
                    xin.append(row)
                return xin

            def proj_group(xin, xg, c, n, m):
                ps = psproj.tile([P, 512], F32, tag="pp")
                for k in range(KD):
                    nc.tensor.matmul(
                        ps[:],
                        wx_sb[k][:, m * P:(m + 1) * P],
                        xin[c][k][:, n * 512:(n + 1) * 512],
                        start=(k == 0), stop=(k == KD - 1),
                    )
                xg_r = xg[c][:].rearrange("p (t m b) -> p t m b", t=TC, m=M8, b=SC)
                ps_r = ps[:].rearrange("p (t b) -> p t b", b=SC)
                for hlf in range(2):
                    dst = xg_r[:, n * 64 + hlf * 32:n * 64 + (hlf + 1) * 32, m, :]
                    src = ps_r[:, hlf * 32:(hlf + 1) * 32, :]
                    if has_bias:
                        nc.vector.tensor_scalar_add(dst, src, gbt[:, m:m + 1])
                    else:
                        nc.vector.tensor_copy(dst, src)

            def alloc_xg():
                return [xgpool.tile([P, TC * GW], BF, tag=f"xg{c}", name=f"xg{c}")
                        for c in range(2)]

            def body():
                for c in range(2):
                    nc.vector.memset(zt[c][:], 0.0)
                    for i in range(2):
                        nc.vector.memset(vt[c][i][:], 0.0)
                prev_h = [None, None]
                xin_c = dma_chunk(0)
                xg_c = alloc_xg()
                for c in range(2):
                    for n in range(WCH // 512):
                        for m in range(M8):
                            proj_group(xin_c, xg_c, c, n, m)

                for ch in range(NCH):
                    nxt = ch + 1
                    tasks = []
                    xin_n = xg_n = None
                    if nxt < NCH:
                        xin_n = dma_chunk(nxt)
                        xg_n = alloc_xg()
                        tasks = [(c, n, m) for c in range(2)
                                 for n in range(WCH // 512) for m in range(M8)]

                    hb = [hpool.tile([P, (TC + 1) * SL], BF, tag=f"hb{c}", name=f"hb{c}")
                          for c in range(2)]
                    for c in range(2):
                        if ch == 0:
                            nc.vector.memset(hb[c][:, 0:SL], 0.0)
                        else:
                            nc.vector.tensor_copy(hb[c][:, 0:SL], prev_h[c])

                    def step(c, tl):
                        ps = psscan.tile([P, GW], F32, tag=f"pg{c}", name=f"pg{c}")
                        # xg lands in PSUM via identity matmul (starts the
                        # accumulation group; depends only on the xg tile)
                        nc.tensor.matmul(
                            ps[:], eye_sb[:],
                            xg_c[c][:, tl * GW:(tl + 1) * GW],
                            start=True, stop=False, skip_group_check=True,
                        )
                        for k in range(KH):
                            for m in range(M8):
                                nc.tensor.matmul(
                                    ps[:, m * SC:(m + 1) * SC],
                                    wh_sb[k][:, m * P:(m + 1) * P],
                                    hb[c][:, tl * SL + k * SC: tl * SL + (k + 1) * SC],
                                    start=False,
                                    stop=(k == KH - 1 and m == M8 - 1),
                                    skip_group_check=True,
                                )
                        va = vt[c][gstep[c] % 2]
                        vb = vt[c][(gstep[c] + 1) % 2]
                        gstep[c] += 1

                        def pv(ap):
                            return ap.rearrange(
                                "p (q two) -> p q two", two=2)[:, :, 0]

                        # one sigmoid over all 4 gates, scattered to even
                        # cols of zt (g weights host-scaled x2)
                        nc.scalar.activation(pv(zt[c][:, 0:8 * SL]), ps[:],
                                             AF.Sigmoid)
                        # p/2 = (sig(2g)-0.5)*sig_i -> even cols 2..32 of va
                        nc.vector.scalar_tensor_tensor(
                            pv(va[:, 2:2 * SL + 2]),
                            pv(zt[c][:, 4 * SL:6 * SL]), 0.5,
                            pv(zt[c][:, 0:2 * SL]),
                            mybir.AluOpType.subtract, mybir.AluOpType.mult,
                        )
                        # c/2 scan: state = f*state + p/2 over [load, update]
                        # pairs; d0 = zt[2*SL-1 : 4*SL-1] = [0, f0, 0, f1,...]
                        nc.vector.tensor_tensor_scan(
                            vb[:, 0:2 * SL], zt[c][:, 2 * SL - 1:4 * SL - 1],
                            va[:, 1:2 * SL + 1], 0.0,
                            mybir.AluOpType.mult, mybir.AluOpType.add,
                        )
                        # tanh(c) = Tanh(2 * c/2); c_new at odd cols of vb
                        nc.scalar.activation(tcc[c][:],
                                             pv(vb[:, 1:2 * SL + 1]),
                                             AF.Tanh, scale=2.0)
                        # h = sig_o * tanh_c in one op: all 16 next-step
                        # MMs release together (the old low/high split cost
                        # a second dependent DVE op for ~no earlier start)
                        so_v = pv(zt[c][:, 6 * SL:8 * SL])
                        nc.vector.tensor_mul(
                            hb[c][:, (tl + 1) * SL:(tl + 2) * SL],
                            so_v, tcc[c][:],
                        )

                    for tl in range(TC):
                        step(0, tl)
                        step(1, tl)
                        if tasks and tl % 4 == 3:
                            c, n, m = tasks.pop(0)
                            proj_group(xin_n, xg_n, c, n, m)

                    for c in range(2):
                        nc.sync.dma_start(
                            out[:, c * T * SL + ch * TC * SL:
                                c * T * SL + (ch + 1) * TC * SL],
                            hb[c][:, SL:],
                        )
                        prev_h[c] = hb[c][:, TC * SL:(TC + 1) * SL]
                    xin_c, xg_c = xin_n, xg_n

            if loop_n is None:
                body()
            else:
                with tc.For_i(0, loop_n, 1):
                    body()

    nc.compile()
    return nc


def _get_compiled(has_bias):
    if has_bias not in _COMPILED:
        _COMPILED[has_bias] = _build_graph(has_bias=has_bias)
    return _COMPILED[has_bias]


def kernel(inputs, input_paddings, bn_scale, bn_bias, bn_mean, bn_var,
           Wx_f, Wh_f, b_f, Wx_b, Wh_b, b_b):
    from concourse import mybir
    from concourse.bass_utils import run_bass_kernel_spmd

    np_bf16 = mybir.dt.np(mybir.dt.bfloat16)
    np_f8 = mybir.dt.np(mybir.dt.float8e4)

    x = np.asarray(inputs, np.float32)
    pad = np.asarray(input_paddings, np.float32)
    keep = 1.0 - pad
    lengths = (T - pad.sum(axis=1)).astype(np.int64)
    idx = (np.arange(T - 1, -1, -1)[None, :] + lengths[:, None]) % T

    inv = ((1.0 + np.asarray(bn_scale, np.float32))
           / np.sqrt(np.asarray(bn_var, np.float32) + EPS))
    beta = np.asarray(bn_bias, np.float32) - np.asarray(bn_mean, np.float32) * inv

    x_bn = (x * inv + beta) * keep[:, :, None]
    x_flip = np.take_along_axis(x_bn, idx[:, :, None].astype(np.int64), axis=1)

    gate_scale = np.ones((H4,), np.float32)
    gate_scale[2 * H:3 * H] = 2.0

    has_bias = bool(np.any(np.asarray(b_f)) or np.any(np.asarray(b_b)))

    def prep_w(Wx, Wh, b):
        wxp = (np.asarray(Wx, np.float32) * gate_scale).astype(np_bf16)
        whp = (np.asarray(Wh, np.float32) * gate_scale).astype(np_f8)
        wx_t = np.stack([wxp[k * P:(k + 1) * P] for k in range(KD)])
        wh_t = np.stack([whp[k * P:(k + 1) * P] for k in range(KH)])
        gb_t = (np.asarray(b, np.float32) * gate_scale).reshape(M8, P).T.copy()
        return wx_t, wh_t, gb_t

    wx_f_t, wh_f_t, gb_f_t = prep_w(Wx_f, Wh_f, b_f)
    wx_b_t, wh_b_t, gb_b_t = prep_w(Wx_b, Wh_b, b_b)
    eye_t = np.eye(P, dtype=np.float32).astype(np_f8)

    in_maps = []
    for core in range(8):
        fwd = core < GROUPS
        g = core % GROUPS
        sl = slice(g * S, (g + 1) * S)
        xs = (x_bn if fwd else x_flip)[sl]                # [16, T, D]
        # chain-major: [2, D, T*8] -> [D, 2*T*8]
        xc = xs.reshape(2, SC, T, D).transpose(3, 0, 2, 1)  # [D, 2, T, 8]
        xTc = np.ascontiguousarray(xc).reshape(D, 2 * T * SC)
        im = dict(
            xT=xTc.astype(np_bf16),
            wx=(wx_f_t if fwd else wx_b_t),
            wh=(wh_f_t if fwd else wh_b_t),
            eye=eye_t,
        )
        if has_bias:
            im["gb"] = gb_f_t if fwd else gb_b_t
        in_maps.append(im)

    nc = _get_compiled(has_bias)
    res = run_bass_kernel_spmd(nc, in_maps, core_ids=list(range(8)))
    global LAST_RESULT
    LAST_RESULT = res

    out_full = np.zeros((B, T, 2 * H), np.float32)
    for core in range(8):
        fwd = core < GROUPS
        g = core % GROUPS
        sl = slice(g * S, (g + 1) * S)
        oc = np.asarray(res.results[core]["out"], dtype=np_bf16).astype(np.float32)
        # [p, c*T*16 + t*16 + k*8 + b] -> [c*8+b, t, k*128+p]
        hs = oc.reshape(P, 2, T, 2, SC).transpose(1, 4, 2, 3, 0).reshape(S, T, 2 * P)
        if fwd:
            out_full[sl, :, 0:H] = hs
        else:
            hs = np.take_along_axis(hs, idx[sl][:, :, None].astype(np.int64), axis=1)
            out_full[sl, :, H:2 * H] = hs
    return out_full

